# revision 18
# baseline (speedup 1.0000x reference)
"""Trainium2 Bass kernel for CuboidLoss (SSD-style multibox loss over K-frame tubes).

Contract: kernel(**inputs) takes FULL numpy inputs and returns the full output
(tuple (loss_l, loss_c) like the reference). Internally shards batch-parallel
over 8 NeuronCores (8 samples per core) and runs one SPMD Bass program.

v5 design (streaming-only device program; memory-regime):
  The device computes the two big memory-bound streams and nothing else:
    1. IoU geometry per (sample, prior, frame): min-form compare
       u = min([-prmin|+prmax], [-gtmin|+gtmax]); d = u_lo+u_hi; dr = relu(d);
       cross = drx*dry; den = (pa+ga) - cross; recip = exp(-ln(den)).
    2. Per-prior class partition function: ssum = sum_c exp(conf_c) via one
       ACT exp + a class-major bf16 add tree (all unit-stride -> DVE 2x mode;
       a tensor_reduce over [128,132,26] measured 1x on HW, so tree it is).
  Device ships cross, recip, ssum (bf16) back; the host (float64) does the
  k-sum (iou = sum_k cross*recip), threshold/argmax matching, hard-negative
  top-k mining, and the O(B * npos) loss terms from the original f32 inputs
  (same accuracy structure as the earlier device version, but with no top-8
  or npos<=2 assumptions).

  ACT-table discipline: Exp and Ln coexist in the 'natural_log_exp_and_others'
  table; get_activation_tables is narrowed (for this build only) so the table
  pass picks that single table -> one ACT_TABLE_LOAD total, zero switches.

  Scheduling: the IoU chains (DVE long pole) are emitted first so they hold
  the low scheduler priorities; the exp-tree work is emitted after and fills
  DVE gaps while the compare stream (gtb broadcasts) is still arriving.

  Engines used: SP/DMA, ACT, DVE only (no PE/PSUM/GPSIMD) -> short semaphore
  teardown.
"""

import numpy as np
import ml_dtypes

import concourse.bass as bass
import concourse.bacc as bacc_mod
import concourse.tile as tile
from concourse import mybir
from concourse import hw_specs as _hw_specs
from concourse.bass_utils import run_bass_kernel_spmd
from concourse.masks import make_identity

BF = ml_dtypes.bfloat16
F32 = mybir.dt.float32
BF16 = mybir.dt.bfloat16
Alu = mybir.AluOpType
Act = mybir.ActivationFunctionType
Ax = mybir.AxisListType

# Problem constants (hardcoded per the harness contract).
B, P, K, C = 64, 8396, 6, 25
NCORES = 8
BL = B // NCORES          # samples per core = 8
NPAIR = BL // 2           # 4 pair iterations, 2 samples each
QC = 66                   # free-dim groups per partition; prior i = p*QC + q
PPAD = 128 * QC           # 8448 padded priors

CW = C * 2 * QC           # 3300 conf cols per pair tile (c, h, q)
GW = 2 * 2 * K * 2 * QC   # 3168 compare cols per pair (mm, xy, k, h, q)
DW = 2 * K * 2 * QC       # 1584 (xy, k, h, q)
XW = K * 2 * QC           # 792  (k, h, q)
SW = 2 * QC               # 132  (h, q)
VARXY, VARWH = 0.1, 0.2
IOU6_THRESH = 3.0         # 6 * 0.5

_NC_CACHE = {}

# --- ACT-table narrowing: force Exp and Ln onto the one table that holds
# both, so the table-load pass emits a single load and no switches. This
# only filters which (real) table the pass may pick; act_func_set ids keep
# their act_info.json positions, so the emitted NEFF is fully valid.
_ORIG_GET_TABLES = _hw_specs.get_activation_tables


def _get_tables_ln_exp(arch):
    tabs = _ORIG_GET_TABLES(arch)
    for name, funcs in tabs.items():
        if name != "natural_log_exp_and_others":
            funcs.discard(Act.Exp)
            funcs.discard(Act.Ln)
    return tabs


bacc_mod.get_activation_tables = _get_tables_ln_exp


def _build_nc():
    """Build the single SPMD Bass program (same for all 8 cores)."""
    nc = bacc_mod.Bacc("TRN2", target_bir_lowering=False)

    # ---- DRAM I/O ----
    conf2_t = nc.dram_tensor("conf2_t", [NPAIR * 128, CW], BF16,
                             kind="ExternalInput")
    gtrow_t = nc.dram_tensor("gtrow_t", [NPAIR, GW], BF16,
                             kind="ExternalInput")
    prgm_t = nc.dram_tensor("prgm_t", [128, GW], BF16, kind="ExternalInput")
    paga2_t = nc.dram_tensor("paga2_t", [128, NPAIR * XW], BF16,
                             kind="ExternalInput")
    crout_t = nc.dram_tensor("crout_t", [128, NPAIR * 2 * XW], BF16,
                             kind="ExternalOutput")
    ssum_t = nc.dram_tensor("ssum_t", [128, NPAIR * SW], BF16,
                            kind="ExternalOutput")

    HXW = XW // 2  # 396: PSUM-bank-sized den chunk (f32, 1584B < 2KB bank)

    with tile.TileContext(nc) as tc:
        with (
            tc.tile_pool(name="consts", bufs=1) as cs,
            tc.tile_pool(name="stream", bufs=3) as st,
            tc.tile_pool(name="work", bufs=2) as wk,
            tc.tile_pool(name="persist", bufs=1) as pe,
            tc.tile_pool(name="psum", bufs=2, space="PSUM") as ps,
        ):
            prgm = cs.tile([128, GW], BF16)
            paga2 = cs.tile([128, NPAIR * XW], BF16)
            identf = cs.tile([128, 128], F32)
            ident = cs.tile([128, 128], BF16)
            negident = cs.tile([128, 128], BF16)
            lnall = pe.tile([128, NPAIR * XW], F32)
            croutall = pe.tile([128, NPAIR * 2 * XW], BF16)
            ssumall = pe.tile([128, NPAIR * SW], BF16)

            confs = []
            gtbs = []
            for ip in range(NPAIR):
                confs.append(st.tile([128, CW], BF16, tag="conf",
                                     name=f"conf{ip}"))
                gtbs.append(st.tile([128, GW], BF16, tag="gtb",
                                    name=f"gtb{ip}"))

            # ---- input DMAs, issued in intended arrival order ----
            # Compare stream leads (it feeds the DVE long pole); each pair's
            # conf follows its gtb so ACT exp + tree fill DVE gaps. gtb0 is
            # split in half so the first MIN starts one transfer earlier.
            def dma_conf(ip):
                nc.sync.dma_start(out=confs[ip],
                                  in_=conf2_t[ip * 128:(ip + 1) * 128, :])

            def dma_gtb(ip, half=None):
                # broadcast the pair's gt row to all partitions (stride-0 src)
                if half is None:
                    lo, n = 0, GW
                else:
                    lo, n = half * DW, DW
                nc.sync.dma_start(
                    out=gtbs[ip][:, lo:lo + n],
                    in_=bass.AP(tensor=gtrow_t, offset=ip * GW + lo,
                                ap=[[0, 128], [1, n]]))

            nc.sync.dma_start(out=prgm, in_=prgm_t[:, :])
            dma_gtb(0)
            dma_conf(0)
            nc.sync.dma_start(out=paga2, in_=paga2_t[:, :])
            dma_gtb(1)
            dma_conf(1)
            dma_gtb(2)
            dma_conf(2)
            dma_gtb(3)
            dma_conf(3)

            # matmul weights built on device (GPSIMD + idle-DVE window):
            # costs no DMA slot and no HWDGE FIFO position
            make_identity(nc, identf[:])
            nc.vector.tensor_scalar(out=ident, in0=identf, scalar1=1.0,
                                    scalar2=None, op0=Alu.mult)
            nc.vector.tensor_scalar(out=negident, in0=identf, scalar1=-1.0,
                                    scalar2=None, op0=Alu.mult)

            # ---- IoU chains first: they own the low scheduler priorities ----
            for ip in range(NPAIR):
                cr = croutall[:, ip * 2 * XW:ip * 2 * XW + XW]
                rc = croutall[:, ip * 2 * XW + XW:(ip + 1) * 2 * XW]
                xs = slice(ip * XW, (ip + 1) * XW)
                u = wk.tile([128, GW], BF16, tag="u", name=f"u{ip}")
                nc.vector.tensor_tensor(out=u, in0=prgm, in1=gtbs[ip],
                                        op=Alu.min)
                d = wk.tile([128, DW], BF16, tag="d", name=f"d{ip}")
                nc.vector.tensor_tensor(out=d, in0=u[:, 0:DW],
                                        in1=u[:, DW:2 * DW], op=Alu.add)
                dr = wk.tile([128, DW], BF16, tag="dr", name=f"dr{ip}")
                nc.vector.tensor_scalar(out=dr, in0=d, scalar1=0.0,
                                        scalar2=None, op0=Alu.max)
                nc.vector.tensor_tensor(out=cr, in0=dr[:, 0:XW],
                                        in1=dr[:, XW:2 * XW], op=Alu.mult)
                if ip == NPAIR - 1:
                    # ship cross3 early; recip3 follows on its own
                    nc.sync.dma_start(
                        out=crout_t[:, ip * 2 * XW:ip * 2 * XW + XW], in_=cr)
                # den = paga - cross accumulated on the (idle) PE into
                # PSUM (I*paga + (-I)*cross), per 396-col bank; the ACT
                # Ln reads PSUM directly. Frees ~0.5us of DVE per pair.
                for hf in range(2):
                    hs = slice(ip * XW + hf * HXW,
                               ip * XW + (hf + 1) * HXW)
                    dp = ps.tile([128, HXW], F32, space="PSUM",
                                 tag=f"dp{hf}", name=f"dp{ip}_{hf}")
                    nc.tensor.matmul(out=dp[:], lhsT=ident,
                                     rhs=paga2[:, hs],
                                     start=True, stop=False)
                    nc.tensor.matmul(out=dp[:], lhsT=negident,
                                     rhs=cr[:, hf * HXW:(hf + 1) * HXW],
                                     start=False, stop=True)
                    nc.scalar.activation(out=lnall[:, hs], in_=dp[:],
                                         func=Act.Ln)
                # recip = exp(-ln(den)); Exp/Ln share one ACT table here
                nc.scalar.activation(out=rc, in_=lnall[:, xs],
                                     func=Act.Exp, scale=-1.0)
                if ip < NPAIR - 1:
                    nc.sync.dma_start(
                        out=crout_t[:, ip * 2 * XW:(ip + 1) * 2 * XW],
                        in_=croutall[:, ip * 2 * XW:(ip + 1) * 2 * XW])
                else:
                    nc.sync.dma_start(
                        out=crout_t[:, ip * 2 * XW + XW:(ip + 1) * 2 * XW],
                        in_=rc)

            # ---- conf partition function: exp + class-major add tree ----
            # Emitted after the chains -> higher scheduler priority values,
            # so these ops fill DVE idle slots while gtb transfers land.
            for ip in range(NPAIR):
                expv = st.tile([128, CW], BF16, tag="expv", name=f"expv{ip}")
                nc.scalar.activation(out=expv, in_=confs[ip], func=Act.Exp)
                L1 = wk.tile([128, 12 * SW], BF16, tag="L1", name=f"L1_{ip}")
                nc.vector.tensor_tensor(out=L1, in0=expv[:, 0:12 * SW],
                                        in1=expv[:, 12 * SW:24 * SW],
                                        op=Alu.add)
                L2 = wk.tile([128, 6 * SW], BF16, tag="L2", name=f"L2_{ip}")
                nc.vector.tensor_tensor(out=L2, in0=L1[:, 0:6 * SW],
                                        in1=L1[:, 6 * SW:12 * SW], op=Alu.add)
                L3 = wk.tile([128, 3 * SW], BF16, tag="L3", name=f"L3_{ip}")
                nc.vector.tensor_tensor(out=L3, in0=L2[:, 0:3 * SW],
                                        in1=L2[:, 3 * SW:6 * SW], op=Alu.add)
                L4 = wk.tile([128, SW], BF16, tag="L4", name=f"L4_{ip}")
                nc.vector.tensor_tensor(out=L4, in0=L3[:, 0:SW],
                                        in1=L3[:, SW:2 * SW], op=Alu.add)
                L5 = wk.tile([128, SW], BF16, tag="L5", name=f"L5_{ip}")
                nc.vector.tensor_tensor(out=L5, in0=L4,
                                        in1=L3[:, 2 * SW:3 * SW], op=Alu.add)
                nc.vector.tensor_tensor(
                    out=ssumall[:, ip * SW:(ip + 1) * SW], in0=L5,
                    in1=expv[:, 24 * SW:25 * SW], op=Alu.add)
                nc.sync.dma_start(out=ssum_t[:, ip * SW:(ip + 1) * SW],
                                  in_=ssumall[:, ip * SW:(ip + 1) * SW])

    nc.compile()
    return nc


def _host_prep(loc_preds, conf_preds, prior_tubes, ground_truth):
    """Host-side input prep (numpy): pad/reorder into device layouts."""
    pr = prior_tubes.reshape(P, K, 4)
    prp = np.empty((PPAD, K, 4), np.float32)
    prp[:P] = pr
    prp[P:] = np.array([-10.0, -10.0, -9.0, -9.0], np.float32)  # far-away pads
    pr128 = prp.reshape(128, QC, K, 4)

    # prgm [128, (mm, xy, k, h, q)] bf16: mm=0 -> -prmin, mm=1 -> +prmax
    t = np.transpose(pr128, (0, 3, 2, 1))              # [p, coord, k, q]
    prgm6 = np.stack([-t[:, 0:2], t[:, 2:4]], axis=1)  # [p, mm, xy, k, q]
    prgm = np.ascontiguousarray(
        np.broadcast_to(prgm6[:, :, :, :, None, :],
                        (128, 2, 2, K, 2, QC))).reshape(128, GW).astype(BF)

    # prior areas, k-major [p, k, q]
    pa = (pr128[..., 2] - pr128[..., 0]) * (pr128[..., 3] - pr128[..., 1])
    paT = np.transpose(pa, (0, 2, 1))                  # [p, k, q]

    gt = ground_truth[:, 1:].reshape(B, K, 4).astype(np.float32)
    ga = ((gt[..., 2] - gt[..., 0]) * (gt[..., 3] - gt[..., 1])).astype(
        np.float32)

    in_maps = []
    for r in range(NCORES):
        sl = slice(r * BL, (r + 1) * BL)
        # conf2 [ip, p, (c, h, q)]; prior pads: c0=+20, rest -20
        confp = np.empty((BL, PPAD, C), np.float32)
        confp[:, P:, :] = -20.0
        confp[:, P:, 0] = 20.0
        confp[:, :P] = conf_preds[sl]
        v = confp.reshape(NPAIR, 2, 128, QC, C)
        conf2 = np.ascontiguousarray(
            v.transpose(0, 2, 4, 1, 3)).reshape(NPAIR * 128, CW).astype(BF)
        # gtrow [ip, (mm, xy, k, h, q)]: mm=0 -> -gtmin, mm=1 -> +gtmax
        g = gt[sl].reshape(NPAIR, 2, K, 4)             # [ip, h, k, coord]
        gl = np.stack([-np.transpose(g[..., 0:2], (0, 3, 2, 1)),
                       np.transpose(g[..., 2:4], (0, 3, 2, 1))],
                      axis=1)                          # [ip, mm, xy, k, h]
        gtrow = np.ascontiguousarray(
            np.broadcast_to(gl[..., None],
                            (NPAIR, 2, 2, K, 2, QC))).reshape(
                                NPAIR, GW).astype(BF)
        # paga2 [p, (ip, k, h, q)] = pa[p,k,q] + ga[s,k]
        ga4 = np.transpose(ga[sl].reshape(NPAIR, 2, K), (0, 2, 1))  # [ip,k,h]
        paga = paT[:, None, :, None, :] + ga4[None, :, :, :, None]
        paga2 = np.ascontiguousarray(paga).reshape(
            128, NPAIR * XW).astype(BF)
        in_maps.append({
            "conf2_t": conf2, "gtrow_t": gtrow, "prgm_t": prgm,
            "paga2_t": paga2,
        })
    return in_maps


def _finalize(outs, loc_preds, conf_preds, prior_tubes, ground_truth):
    """Host float64 finalize: matching, mining, and both losses from the
    device-computed cross/recip/ssum plus the original f32 inputs."""
    gt_cls = ground_truth[:, 0].astype(np.int32)

    pr = prior_tubes.reshape(P, K, 4).astype(np.float64)
    pcx = (pr[:, :, 0] + pr[:, :, 2]) * 0.5
    pcy = (pr[:, :, 1] + pr[:, :, 3]) * 0.5
    pw = pr[:, :, 2] - pr[:, :, 0]
    ph = pr[:, :, 3] - pr[:, :, 1]
    gt = ground_truth[:, 1:].reshape(B, K, 4).astype(np.float64)
    gcx = (gt[:, :, 0] + gt[:, :, 2]) * 0.5
    gcy = (gt[:, :, 1] + gt[:, :, 3]) * 0.5
    gw = gt[:, :, 2] - gt[:, :, 0]
    gh = gt[:, :, 3] - gt[:, :, 1]
    x0 = conf_preds[:, :, 0].astype(np.float64)        # [B, P]

    n_tot = 0
    sl1s = poslse = xcls = ceneg = 0.0
    for r, m in enumerate(outs):
        crout = np.asarray(m["crout_t"], np.float64).reshape(128, NPAIR, 2,
                                                             K, 2, QC)
        cross = crout[:, :, 0]                         # [128, ip, k, h, q]
        recip = crout[:, :, 1]
        # iou6[s_local, prior] with prior = part*QC + q
        iou6 = np.ascontiguousarray(
            (cross * recip).sum(axis=2).transpose(1, 2, 0, 3)).reshape(
                BL, PPAD)
        ssum8 = np.ascontiguousarray(
            np.asarray(m["ssum_t"], np.float64).reshape(
                128, NPAIR, 2, QC).transpose(1, 2, 0, 3)).reshape(BL, PPAD)
        for sl_ in range(BL):
            s = r * BL + sl_
            v = iou6[sl_]
            thr = min(v.max(), IOU6_THRESH)
            if thr > 0.0:
                pos = v >= thr
                pos[P:] = False
                idx = np.nonzero(pos)[0]
            else:
                idx = np.array([int(np.argmax(v[:P]))])
            npos = len(idx)
            n_tot += npos

            # ---- localization smooth-L1 on positives ----
            lp = loc_preds[s, idx].astype(np.float64)           # [npos, 4K]
            enc = np.empty((npos, K, 4))
            enc[:, :, 0] = (gcx[s][None] - pcx[idx]) / pw[idx] / VARXY
            enc[:, :, 1] = (gcy[s][None] - pcy[idx]) / ph[idx] / VARXY
            enc[:, :, 2] = np.log(gw[s][None] / pw[idx]) / VARWH
            enc[:, :, 3] = np.log(gh[s][None] / ph[idx]) / VARWH
            diff = np.abs(lp - enc.reshape(npos, 4 * K))
            sl1s += np.where(diff < 1.0, 0.5 * diff * diff, diff - 0.5).sum()

            # ---- positive cross-entropy ----
            row = conf_preds[s, idx].astype(np.float64)         # [npos, C]
            poslse += np.log(np.exp(row).sum(axis=1)).sum()
            xcls += row[:, gt_cls[s]].sum()

            # ---- hard-negative mining: top 3*npos scores, positives out ----
            score = ssum8[sl_, :P] * np.exp(-x0[s])
            score[idx] = -np.inf
            kneg = 3 * npos
            top = np.partition(score, P - kneg)[P - kneg:]
            ceneg += np.log(top).sum()

    loss_l = sl1s / K / n_tot
    loss_c = (poslse - xcls + ceneg) / (4.0 * n_tot)
    return np.float32(loss_l), np.float32(loss_c)


def kernel(loc_preds, conf_preds, prior_tubes, ground_truth):
    loc_preds = np.asarray(loc_preds, np.float32)
    conf_preds = np.asarray(conf_preds, np.float32)
    prior_tubes = np.asarray(prior_tubes, np.float32)
    ground_truth = np.asarray(ground_truth, np.float32)

    in_maps = _host_prep(loc_preds, conf_preds, prior_tubes, ground_truth)
    if "nc" not in _NC_CACHE:
        _NC_CACHE["nc"] = _build_nc()
    nc = _NC_CACHE["nc"]
    res = run_bass_kernel_spmd(nc, in_maps, core_ids=list(range(NCORES)))
    return _finalize(res.results, loc_preds, conf_preds, prior_tubes,
                     ground_truth)
